# revision 1
# baseline (speedup 1.0000x reference)
"""Trainium2 Bass kernel for nn_Expand_36610301231376.

kernel(**inputs) takes the FULL unsharded inputs (as in reference.setup_inputs)
and returns the FULL (16, 512, 56, 56) float32 output.

Strategy: pure data parallel over batch B=16 across 8 NeuronCores (2 batches
per core); all parameters replicated. Inside each core, tokens (56x56=3136)
are processed channel-major in 7 chunks of 448 (8 image rows); row-local
attention runs on 2-row blocks of 112 tokens. All matmuls are bf16 with fp32
PSUM accumulation; LayerNorm statistics are computed with ones-matmuls on the
TensorEngine and rsqrt on (1,448) stat rows, then broadcast back via a PE
rank-1 matmul. The conv bias b_conv1 cancels exactly in LayerNorm and the
1/sqrt(512) attention scale, LN gammas/betas, positional encodings and
projection biases are folded into host-precomputed constants.

The two batches of each chunk are software-pipelined (phase-interleaved
emission) so the TensorEngine stays busy through the elementwise phases.
"""
import sys

if "/opt/trn_rl_repo" not in sys.path:
    sys.path.insert(0, "/opt/trn_rl_repo")

import numpy as np
import orjson

# ----------------------------------------------------------------------------
# BIR post-pass: this container's walrus build supports only ONE sync-wait per
# instruction; split multi-wait instructions into single-wait NoOps.
# ----------------------------------------------------------------------------
_wcounter = [0]


def _split_block(instructions):
    out, changed = [], False
    for inst in instructions:
        si = inst.get("sync_info")
        waits = (si or {}).get("on_wait") or []
        if len(waits) > 1:
            changed = True
            for w in waits[:-1]:
                _wcounter[0] += 1
                nop = {
                    "engine": inst["engine"], "ins": [], "outs": [],
                    "name": f"I-wsplit-{_wcounter[0]}", "opcode": "NoOp",
                    "sync_info": {"on_update": [], "on_wait": [w]},
                }
                if "debug" in inst:
                    nop["debug"] = inst["debug"]
                out.append(nop)
            si["on_wait"] = [waits[-1]]
        out.append(inst)
    return out, changed


def _split_multi_waits_json(bir_json: bytes) -> bytes:
    m = orjson.loads(bir_json)
    changed = False
    for fn in m.get("functions", []):
        for blk in fn.get("blocks", []):
            insts = blk.get("instructions")
            if insts:
                blk["instructions"], ch = _split_block(insts)
                changed = changed or ch
    return orjson.dumps(m) if changed else bir_json


def _install_patch():
    import concourse.bass as bass

    if getattr(bass.Bass, "_wait_split_installed", False):
        return
    orig = bass.Bass.to_json_bytes

    def to_json_bytes(self):
        return _split_multi_waits_json(orig(self))

    bass.Bass.to_json_bytes = to_json_bytes
    bass.Bass._wait_split_installed = True


# ----------------------------------------------------------------------------
# Problem constants (hardcoded from the problem spec)
# ----------------------------------------------------------------------------
B = 16
N_CORES = 8
B_LOC = B // N_CORES
T_LEN, T_DIM = 149, 768
H = W = 56
S_DIM = 512
N_TOK = H * W           # 3136
CH = 448                # tokens per chunk (8 image rows)
NCHUNK = N_TOK // CH    # 7
NBLK = CH // 112        # 4 two-row attention blocks per chunk
EPS = 1e-5


# ----------------------------------------------------------------------------
# Device program
# ----------------------------------------------------------------------------
def _build_program():
    import concourse.bass as bass
    import concourse.tile as tile
    from concourse import mybir

    F32 = mybir.dt.float32
    BF16 = mybir.dt.bfloat16
    AF = mybir.ActivationFunctionType
    OP = mybir.AluOpType

    nc = bass.Bass(trn_type="TRN2", target_bir_lowering=False, debug=False)
    din = {}
    for name, shape, dt_ in [
        ("x0", (128, B_LOC, T_DIM), BF16), ("x1", (32, B_LOC, T_DIM), BF16),
        ("w1t", (128, 2, N_TOK), BF16),
        ("wqgt", (128, 6, S_DIM), BF16), ("uq", (1, S_DIM), BF16),
        ("wkt", (128, 4, S_DIM), BF16),
        ("ones", (128, 128), BF16), ("ident", (128, 128), BF16),
        ("cq", (128, 4, N_TOK), F32), ("bks", (128, 4), F32),
        ("g2", (128, 4), F32), ("pe2p", (128, 4, N_TOK), F32),
        ("masks", (112, 112), F32),
        ("y", (B_LOC, 128, 4, N_TOK), F32),
    ]:
        din[name] = nc.dram_tensor(name, list(shape), dt_, kind="ExternalInput").ap()
    dout = nc.dram_tensor("out", [B_LOC, 128, 4, N_TOK], F32,
                          kind="ExternalOutput").ap()

    from contextlib import ExitStack

    with nc.allow_low_precision(reason="bf16 matmul operands, fp32 accumulate"), \
         tile.TileContext(nc) as tc, ExitStack() as ctx:
        singles = ctx.enter_context(tc.tile_pool(name="singles", bufs=1))
        io3 = ctx.enter_context(tc.tile_pool(name="io3", bufs=3))
        io2 = ctx.enter_context(tc.tile_pool(name="io2", bufs=2))
        wk2 = ctx.enter_context(tc.tile_pool(name="wk2", bufs=2))
        att = ctx.enter_context(tc.tile_pool(name="att", bufs=3))
        sc = ctx.enter_context(tc.tile_pool(name="sc", bufs=2))
        ps_mm = ctx.enter_context(tc.tile_pool(name="ps_mm", bufs=4, space="PSUM"))
        ps_att = ctx.enter_context(tc.tile_pool(name="ps_att", bufs=2, space="PSUM"))
        ps_st = ctx.enter_context(tc.tile_pool(name="ps_st", bufs=2, space="PSUM"))

        def load(name, shape, dt_):
            t = singles.tile(list(shape), dt_, tag=name)
            nc.sync.dma_start(out=t, in_=din[name])
            return t

        x0 = load("x0", (128, B_LOC, T_DIM), BF16)
        x1 = load("x1", (32, B_LOC, T_DIM), BF16)
        w1t = load("w1t", (128, 2, N_TOK), BF16)
        wqgt = load("wqgt", (128, 6, S_DIM), BF16)
        uq = load("uq", (1, S_DIM), BF16)
        wkt = load("wkt", (128, 4, S_DIM), BF16)
        ones = load("ones", (128, 128), BF16)
        ident = load("ident", (128, 128), BF16)
        bks = load("bks", (128, 4), F32)
        g2 = load("g2", (128, 4), F32)
        masks = load("masks", (112, 112), F32)
        ones_col = ones[:, 0:1]
        ones_row = ones[0:1, :]
        eps1 = singles.tile([1, 1], F32)
        nc.vector.memset(eps1, EPS)

        x_k = [(x0, 128), (x1, 21)]

        def phase_load(st):
            b, cols = st["b"], st["cols"]
            y_t = io3.tile([128, 4, CH], F32, tag="y")
            nc.sync.dma_start(out=y_t, in_=din["y"][b, :, :, cols])
            ybf = wk2.tile([128, 4, CH], BF16, tag="ybf")
            nc.scalar.activation(out=ybf, in_=y_t, func=AF.Copy)
            st["y_t"], st["ybf"] = y_t, ybf

        def phase_xe(st):
            b, cols = st["b"], st["cols"]
            xe = wk2.tile([128, 6, CH], BF16, tag="xe")
            sq = wk2.tile([128, 6, CH], BF16, tag="sq")
            for m in range(6):
                pxe = ps_mm.tile([128, CH], F32, tag="mm")
                for ik, (xt, kv) in enumerate(x_k):
                    nc.tensor.matmul(
                        pxe, xt[:kv, b, m * 128:(m + 1) * 128],
                        w1t[:kv, ik, cols], start=(ik == 0), stop=(ik == 1))
                if m % 2 == 0:
                    nc.vector.tensor_copy(out=xe[:, m, :], in_=pxe)
                else:
                    nc.scalar.activation(out=xe[:, m, :], in_=pxe, func=AF.Copy)
                nc.scalar.square(out=sq[:, m, :], in_=pxe)
            ps1 = ps_st.tile([1, CH], F32, tag="st")
            for m in range(6):
                nc.tensor.matmul(ps1, ones_col, xe[:, m, :],
                                 start=(m == 0), stop=(m == 5))
            pq1 = ps_st.tile([1, CH], F32, tag="st")
            for m in range(6):
                nc.tensor.matmul(pq1, ones_col, sq[:, m, :],
                                 start=(m == 0), stop=(m == 5))
            st["xe"], st["ps1"], st["pq1"] = xe, ps1, pq1

        def _rsqrt_row(psum_s, psum_q, inv_d):
            mrow = sc.tile([1, CH], BF16, tag="ma")
            nc.scalar.activation(out=mrow, in_=psum_s, func=AF.Copy, scale=-inv_d)
            vrow = sc.tile([1, CH], F32, tag="vb")
            nc.vector.tensor_scalar_mul(out=vrow, in0=psum_q, scalar1=inv_d)
            t = sc.tile([1, CH], F32, tag="t")
            nc.vector.tensor_mul(out=t, in0=mrow, in1=mrow)
            nc.vector.tensor_tensor(out=vrow, in0=vrow, in1=t, op=OP.subtract)
            nc.scalar.activation(out=vrow, in_=vrow, func=AF.Sqrt, bias=eps1)
            rrow = sc.tile([1, CH], BF16, tag="rr")
            nc.vector.reciprocal(out=rrow, in_=vrow)
            return mrow, rrow

        def phase_stats1(st):
            mrow1, rrow1 = _rsqrt_row(st["ps1"], st["pq1"], 1.0 / T_DIM)
            prb = ps_st.tile([128, CH], F32, tag="st")
            nc.tensor.matmul(prb, ones_row, rrow1, start=True, stop=True)
            r1b = wk2.tile([128, CH], F32, tag="r1b")
            nc.vector.tensor_copy(out=r1b, in_=prb)
            st["mrow1"], st["r1b"] = mrow1, r1b

        def phase_q(st):
            xe, mrow1, r1b = st["xe"], st["mrow1"], st["r1b"]
            cq_t = st["cq_t"]
            q = wk2.tile([128, 4, CH], BF16, tag="q")
            for oc in range(4):
                pq = ps_mm.tile([128, CH], F32, tag="mm")
                for kc in range(6):
                    nc.tensor.matmul(
                        pq, wqgt[:, kc, oc * 128:(oc + 1) * 128],
                        xe[:, kc, :], start=(kc == 0), stop=False)
                nc.tensor.matmul(pq, uq[:, oc * 128:(oc + 1) * 128], mrow1,
                                 start=False, stop=True)
                nc.vector.tensor_mul(out=q[:, oc, :], in0=pq, in1=r1b)
                nc.vector.tensor_add(out=q[:, oc, :], in0=q[:, oc, :],
                                     in1=cq_t[:, oc, :])
            st["q"] = q

        def phase_stats2a(st):
            ybf = st["ybf"]
            sq2 = wk2.tile([128, 6, CH], BF16, tag="sq")
            nc.scalar.square(out=sq2[:, 0:4, :], in_=ybf)
            ps2 = ps_st.tile([1, CH], F32, tag="st")
            for m in range(4):
                nc.tensor.matmul(ps2, ones_col, ybf[:, m, :],
                                 start=(m == 0), stop=(m == 3))
            pq2 = ps_st.tile([1, CH], F32, tag="st")
            for m in range(4):
                nc.tensor.matmul(pq2, ones_col, sq2[:, m, :],
                                 start=(m == 0), stop=(m == 3))
            st["ps2"], st["pq2"] = ps2, pq2

        def phase_stats2b(st):
            mrow2, rrow2 = _rsqrt_row(st["ps2"], st["pq2"], 1.0 / S_DIM)
            pmb2 = ps_st.tile([128, CH], F32, tag="st")
            nc.tensor.matmul(pmb2, ones_row, mrow2, start=True, stop=True)
            prb2 = ps_st.tile([128, CH], F32, tag="st")
            nc.tensor.matmul(prb2, ones_row, rrow2, start=True, stop=True)
            r2b = wk2.tile([128, CH], F32, tag="r2b")
            nc.vector.tensor_copy(out=r2b, in_=prb2)
            c2b = wk2.tile([128, CH], F32, tag="c2b")
            nc.vector.tensor_tensor(out=c2b, in0=pmb2, in1=r2b, op=OP.mult)
            st["r2b"], st["c2b"] = r2b, c2b

        def phase_ny(st):
            y_t, r2b, c2b, pe2_t = st["y_t"], st["r2b"], st["c2b"], st["pe2_t"]
            ny = wk2.tile([128, 4, CH], BF16, tag="ny")
            nyf = wk2.tile([128, 4, CH], F32, tag="nyf")
            for co in range(4):
                nc.vector.tensor_mul(out=nyf[:, co, :], in0=y_t[:, co, :], in1=r2b)
                nc.gpsimd.tensor_add(out=nyf[:, co, :], in0=nyf[:, co, :], in1=c2b)
                nc.scalar.activation(out=nyf[:, co, :], in_=nyf[:, co, :],
                                     func=AF.Identity, scale=g2[:, co:co + 1])
                nc.vector.tensor_tensor(out=ny[:, co, :], in0=nyf[:, co, :],
                                        in1=pe2_t[:, co, :], op=OP.add)
            st["ny"] = ny

        def phase_kv(st):
            ny = st["ny"]
            k = wk2.tile([128, 4, CH], BF16, tag="k")
            for oc in range(4):
                pk = ps_mm.tile([128, CH], F32, tag="mm")
                for kc in range(4):
                    nc.tensor.matmul(
                        pk, wkt[:, kc, oc * 128:(oc + 1) * 128],
                        ny[:, kc, :], start=(kc == 0), stop=(kc == 3))
                nc.vector.tensor_scalar(out=k[:, oc, :], in0=pk,
                                        scalar1=bks[:, oc:oc + 1],
                                        scalar2=None, op0=OP.add)
            v = wk2.tile([112, 4, S_DIM], BF16, tag="v")
            for blk in range(NBLK):
                tb = slice(blk * 112, (blk + 1) * 112)
                for co in range(4):
                    pt = ps_att.tile([112, 128], BF16, tag="at")
                    nc.tensor.transpose(pt, ny[:, co, tb], ident)
                    if co % 2 == 0:
                        nc.vector.tensor_copy(
                            out=v[:, blk, co * 128:(co + 1) * 128], in_=pt)
                    else:
                        nc.scalar.activation(
                            out=v[:, blk, co * 128:(co + 1) * 128], in_=pt,
                            func=AF.Copy)
            st["k"], st["v"] = k, v

        def phase_att(st):
            b, cols = st["b"], st["cols"]
            q, k, v, y_t = st["q"], st["k"], st["v"], st["y_t"]
            out_t = io2.tile([128, 4, CH], F32, tag="out")
            for blk in range(NBLK):
                tb = slice(blk * 112, (blk + 1) * 112)
                psc = ps_att.tile([112, 112], F32, tag="at")
                for oc in range(4):
                    nc.tensor.matmul(psc, q[:, oc, tb], k[:, oc, tb],
                                     start=(oc == 0), stop=(oc == 3))
                e_t = att.tile([112, 112], F32, tag="e")
                nc.vector.tensor_add(out=e_t, in0=psc, in1=masks)
                den = att.tile([112, 1], F32, tag="den")
                nc.scalar.activation(out=e_t, in_=e_t, func=AF.Exp, accum_out=den)
                nc.vector.reciprocal(out=den, in_=den)
                attn = att.tile([112, 112], BF16, tag="attn")
                nc.vector.tensor_scalar_mul(out=attn, in0=e_t, scalar1=den)
                pat = ps_att.tile([112, 112], BF16, tag="at")
                nc.tensor.transpose(pat, attn, ident[:112, :112])
                attnT = att.tile([112, 112], BF16, tag="attnT")
                nc.vector.tensor_copy(out=attnT, in_=pat)
                pav = ps_mm.tile([128, 4, 112], F32, tag="mm")
                for co in range(4):
                    nc.tensor.matmul(pav[:, co, :],
                                     v[:, blk, co * 128:(co + 1) * 128],
                                     attnT, start=True, stop=True)
                nc.vector.tensor_add(out=out_t[:, :, tb], in0=pav,
                                     in1=y_t[:, :, tb])
            nc.sync.dma_start(out=dout[b, :, :, cols], in_=out_t)

        for ich in range(NCHUNK):
            cols = slice(ich * CH, (ich + 1) * CH)
            cq_t = io2.tile([128, 4, CH], F32, tag="cq")
            nc.sync.dma_start(out=cq_t, in_=din["cq"][:, :, cols])
            pe2_t = io2.tile([128, 4, CH], F32, tag="pe2")
            nc.sync.dma_start(out=pe2_t, in_=din["pe2p"][:, :, cols])

            s0 = {"b": 0, "cols": cols, "cq_t": cq_t, "pe2_t": pe2_t}
            s1 = {"b": 1, "cols": cols, "cq_t": cq_t, "pe2_t": pe2_t}

            phase_load(s0)
            phase_xe(s0)
            phase_load(s1)
            phase_stats1(s0)
            phase_stats2a(s0)
            phase_xe(s1)
            phase_stats2b(s0)
            phase_q(s0)
            phase_stats1(s1)
            phase_ny(s0)
            phase_stats2a(s1)
            phase_stats2b(s1)
            phase_q(s1)
            phase_kv(s0)
            phase_ny(s1)
            phase_att(s0)
            phase_kv(s1)
            phase_att(s1)
    return nc


# ----------------------------------------------------------------------------
# Host-side preparation
# ----------------------------------------------------------------------------
def _make_const_inputs(W_conv1, b_conv1, ln1_g, ln1_b, ln2_g, ln2_b,
                       pe_wave, pe_spec, Wq, bq, Wk, bk):
    import ml_dtypes
    f = np.float32
    bf = ml_dtypes.bfloat16
    s = np.float32(S_DIM) ** np.float32(-0.25)

    w1t = np.zeros((128, 2, N_TOK), dtype=f)
    w1T = W_conv1.T.astype(f)
    w1t[:, 0, :] = w1T[:128]
    w1t[:21, 1, :] = w1T[128:]

    wqg = (Wq * ln1_g[None, :]).astype(f) * s
    wqgt = wqg.T.reshape(6, 128, S_DIM).transpose(1, 0, 2).copy()
    uq = (Wq @ ln1_g).astype(f)[None, :] * s

    pe_w = pe_wave.reshape(T_DIM, N_TOK).astype(f)
    cq = (Wq @ (ln1_b[:, None] + pe_w)).astype(f) * s + (bq[:, None] * s).astype(f)
    cq = cq.reshape(4, 128, N_TOK).transpose(1, 0, 2).copy()

    wkt = (Wk.T * s).astype(f).reshape(4, 128, S_DIM).transpose(1, 0, 2).copy()
    bks = (bk * s).astype(f).reshape(4, 128).T.copy()
    g2 = ln2_g.astype(f).reshape(4, 128).T.copy()

    pe2p = (pe_spec.reshape(S_DIM, N_TOK) + ln2_b[:, None]).astype(f)
    pe2p = pe2p.reshape(4, 128, N_TOK).transpose(1, 0, 2).copy()

    masks = np.full((112, 112), -1e30, dtype=f)
    for sb in range(2):
        masks[sb * 56:(sb + 1) * 56, sb * 56:(sb + 1) * 56] = 0.0

    return {
        "w1t": w1t.astype(bf), "wqgt": wqgt.astype(bf), "uq": uq.astype(bf),
        "cq": cq, "wkt": wkt.astype(bf), "bks": bks, "g2": g2,
        "pe2p": pe2p, "masks": masks,
        "ones": np.ones((128, 128), dtype=bf),
        "ident": np.eye(128, dtype=bf),
    }


def _make_core_inputs(consts, x_shard, y_shard):
    import ml_dtypes
    f = np.float32
    bf = ml_dtypes.bfloat16
    x0 = x_shard[:, :128, :].transpose(1, 0, 2).astype(bf).copy()
    x1 = np.zeros((32, B_LOC, T_DIM), dtype=bf)
    x1[:21] = x_shard[:, 128:, :].transpose(1, 0, 2).astype(bf)
    y = y_shard.reshape(B_LOC, 4, 128, N_TOK).transpose(0, 2, 1, 3).astype(f).copy()
    m = {"x0": x0, "x1": x1, "y": y}
    m.update(consts)
    return m


_cached_nc = [None]


def kernel(x, y, W_conv1, b_conv1, ln1_g, ln1_b, ln2_g, ln2_b,
           pe_wave, pe_spec, Wq, bq, Wk, bk):
    _install_patch()
    from concourse.bass_utils import run_bass_kernel_spmd

    x = np.asarray(x, dtype=np.float32)
    y = np.asarray(y, dtype=np.float32)
    consts = _make_const_inputs(
        np.asarray(W_conv1, np.float32), np.asarray(b_conv1, np.float32),
        np.asarray(ln1_g, np.float32), np.asarray(ln1_b, np.float32),
        np.asarray(ln2_g, np.float32), np.asarray(ln2_b, np.float32),
        np.asarray(pe_wave, np.float32), np.asarray(pe_spec, np.float32),
        np.asarray(Wq, np.float32), np.asarray(bq, np.float32),
        np.asarray(Wk, np.float32), np.asarray(bk, np.float32))
    in_maps = [
        _make_core_inputs(consts, x[B_LOC * i:B_LOC * (i + 1)],
                          y[B_LOC * i:B_LOC * (i + 1)])
        for i in range(N_CORES)
    ]

    if _cached_nc[0] is None:
        _cached_nc[0] = _build_program()
    nc = _cached_nc[0]

    res = run_bass_kernel_spmd(nc, in_maps, core_ids=list(range(N_CORES)))
    outs = []
    for i in range(N_CORES):
        o = res.results[i]["out"]  # (B_LOC, 128, 4, N_TOK)
        outs.append(o.transpose(0, 2, 1, 3).reshape(B_LOC, S_DIM, H, W))
    return np.concatenate(outs, axis=0).astype(np.float32)



# revision 34
# speedup vs baseline: 1.9847x; 1.9847x over previous
"""Trainium2 Bass kernel for nn_Expand_36610301231376.

kernel(**inputs) takes the FULL unsharded inputs (as in reference.setup_inputs)
and returns the FULL (16, 512, 56, 56) float32 output.

Strategy: pure data parallel over batch B=16 across 8 NeuronCores (2 batches
per core). Key algebraic restructurings vs a direct implementation:

- The q-projection is composed through the rank-149 conv bottleneck:
  Bq = x @ (diag(g1) Wq^T)  [149, 512] per batch (cheap), then
  q_raw^T = Bq^T routed through W1^T per token chunk (2 matmuls per 128-wide
  output chunk instead of 6), eliminating the xe = conv1(x) materialization.
- LN1 statistics come from the tiny Gram matrix G = x x^T [149,149] and the
  row-sum s_x = sum_d x: sum_d xe = W1 @ s_x, sum_d xe^2 = diag(W1 G W1^T),
  computed per chunk with 8 small matmuls instead of 12 full reductions plus
  squares.
- LN mean/rsqrt chains for both layernorms are batched into one [2,448] pass
  per chunk-batch; reciprocals use the fast approximate DVE op.
- The conv bias b_conv1 cancels in LN; LN gammas/betas, positional encodings,
  projection biases and the 1/sqrt(512) attention scale are folded into
  host-precomputed constants.
- The residual +y is applied on the host; the device returns bf16 attention
  output, halving output DMA.
"""
import sys

if "/opt/trn_rl_repo" not in sys.path:
    sys.path.insert(0, "/opt/trn_rl_repo")

import numpy as np
import orjson

# ----------------------------------------------------------------------------
# BIR post-pass: this container's walrus build supports only ONE sync-wait per
# instruction; split multi-wait instructions into single-wait NoOps.
# ----------------------------------------------------------------------------
_wcounter = [0]


def _split_block(instructions):
    out, changed = [], False
    for inst in instructions:
        si = inst.get("sync_info")
        waits = (si or {}).get("on_wait") or []
        if len(waits) > 1:
            changed = True
            for w in waits[:-1]:
                _wcounter[0] += 1
                nop = {
                    "engine": inst["engine"], "ins": [], "outs": [],
                    "name": f"I-wsplit-{_wcounter[0]}", "opcode": "NoOp",
                    "sync_info": {"on_update": [], "on_wait": [w]},
                }
                if "debug" in inst:
                    nop["debug"] = inst["debug"]
                out.append(nop)
            si["on_wait"] = [waits[-1]]
        out.append(inst)
    return out, changed


def _split_multi_waits_json(bir_json: bytes) -> bytes:
    m = orjson.loads(bir_json)
    changed = False
    for fn in m.get("functions", []):
        for blk in fn.get("blocks", []):
            insts = blk.get("instructions")
            if insts:
                blk["instructions"], ch = _split_block(insts)
                changed = changed or ch
    return orjson.dumps(m) if changed else bir_json


def _install_patch():
    import concourse.bass as bass

    if getattr(bass.Bass, "_wait_split_installed", False):
        return
    orig = bass.Bass.to_json_bytes

    def to_json_bytes(self):
        return _split_multi_waits_json(orig(self))

    bass.Bass.to_json_bytes = to_json_bytes
    bass.Bass._wait_split_installed = True


# ----------------------------------------------------------------------------
# Problem constants (hardcoded from the problem spec)
# ----------------------------------------------------------------------------
B = 16
N_CORES = 8
B_LOC = B // N_CORES
T_LEN, T_DIM = 149, 768
H = W = 56
S_DIM = 512
N_TOK = H * W           # 3136
CH = 448                # tokens per chunk (8 image rows)
NCHUNK = N_TOK // CH    # 7
NBLK = CH // 112        # 4 two-row attention blocks per chunk
EPS = 1e-5


# ----------------------------------------------------------------------------
# Device program
# ----------------------------------------------------------------------------
def _build_program(apply_g2: bool):
    import concourse.bass as bass
    import concourse.tile as tile
    from concourse import mybir

    F32 = mybir.dt.float32
    BF16 = mybir.dt.bfloat16
    AF = mybir.ActivationFunctionType
    OP = mybir.AluOpType

    nc = bass.Bass(trn_type="TRN2", target_bir_lowering=False, debug=False)
    din = {}
    for name, shape, dt_ in [
        ("x0", (128, B_LOC, T_DIM), BF16), ("x1", (32, B_LOC, T_DIM), BF16),
        ("xdl", (128, 6, B_LOC, T_LEN), BF16),
        ("wqg", (128, 6, S_DIM), BF16),
        ("w1t", (128, 2, N_TOK), BF16),
        ("uq", (1, S_DIM), BF16),
        ("wkt", (128, 4, S_DIM), BF16),
        ("ones", (128, 128), BF16), ("ident", (128, 128), BF16),
        ("cq", (128, 4, N_TOK), BF16), ("bks", (128, 4), F32),
        ("g2", (128, 4), F32), ("pe2p", (128, 4, N_TOK), BF16),
        ("masks", (112, 112), BF16),
        ("icol", (128, 2), BF16),
        ("ybf", (B_LOC, 128, 4, N_TOK), BF16),
    ]:
        din[name] = nc.dram_tensor(name, list(shape), dt_, kind="ExternalInput").ap()
    dout = nc.dram_tensor("out", [B_LOC, 128, 4, N_TOK], BF16,
                          kind="ExternalOutput").ap()

    from contextlib import ExitStack

    with nc.allow_low_precision(reason="bf16 matmul operands, fp32 accumulate"), \
         tile.TileContext(nc) as tc, ExitStack() as ctx:
        singles = ctx.enter_context(tc.tile_pool(name="singles", bufs=1))
        io3 = ctx.enter_context(tc.tile_pool(name="io3", bufs=3))
        io2 = ctx.enter_context(tc.tile_pool(name="io2", bufs=2))
        wk2 = ctx.enter_context(tc.tile_pool(name="wk2", bufs=2))
        att = ctx.enter_context(tc.tile_pool(name="att", bufs=3))
        sc = ctx.enter_context(tc.tile_pool(name="sc", bufs=2))
        ps_mm = ctx.enter_context(tc.tile_pool(name="ps_mm", bufs=3, space="PSUM"))
        ps_st = ctx.enter_context(tc.tile_pool(name="ps_st", bufs=2, space="PSUM"))
        ps_att = ctx.enter_context(tc.tile_pool(name="ps_att", bufs=2, space="PSUM"))

        def load(name, shape, dt_):
            t = singles.tile(list(shape), dt_, tag=name)
            nc.sync.dma_start(out=t, in_=din[name])
            return t

        x0 = load("x0", (128, B_LOC, T_DIM), BF16)
        x1 = load("x1", (32, B_LOC, T_DIM), BF16)
        xdl = load("xdl", (128, 6, B_LOC, T_LEN), BF16)
        wqg = load("wqg", (128, 6, S_DIM), BF16)
        w1t = load("w1t", (128, 2, N_TOK), BF16)
        uq = load("uq", (1, S_DIM), BF16)
        wkt = load("wkt", (128, 4, S_DIM), BF16)
        ones = load("ones", (128, 128), BF16)
        ident = load("ident", (128, 128), BF16)
        bks = load("bks", (128, 4), F32)
        g2 = load("g2", (128, 4), F32)
        masks = load("masks", (112, 112), BF16)
        ones_col = ones[:, 0:1]
        ones_row = ones[0:1, :]
        eps1 = singles.tile([1, 1], F32)
        nc.vector.memset(eps1, EPS)
        icol = load("icol", (128, 2), BF16)

        # ---- per-batch preamble: s_x, Bq = x @ Wqg, G = x x^T ----
        pre = {}
        for b in range(B_LOC):
            sx0 = singles.tile([128, 1], F32, tag=f"sx0_{b}")
            nc.vector.tensor_reduce(out=sx0, in_=x0[:, b, :],
                                    axis=mybir.AxisListType.X, op=OP.add)
            sx1 = singles.tile([32, 1], F32, tag=f"sx1_{b}")
            nc.vector.tensor_reduce(out=sx1, in_=x1[:, b, :],
                                    axis=mybir.AxisListType.X, op=OP.add)
            sxb0 = singles.tile([128, 1], BF16, tag=f"sxb0_{b}")
            nc.scalar.activation(out=sxb0, in_=sx0, func=AF.Copy,
                                 scale=1.0 / T_DIM)
            sxb1 = singles.tile([32, 1], BF16, tag=f"sxb1_{b}")
            nc.scalar.activation(out=sxb1, in_=sx1, func=AF.Copy,
                                 scale=1.0 / T_DIM)

            bq_ps0 = ps_mm.tile([128, S_DIM], F32, tag="mm")
            for dc in range(6):
                nc.tensor.matmul(bq_ps0, xdl[:, dc, b, 0:128], wqg[:, dc, :],
                                 start=(dc == 0), stop=(dc == 5))
            bq0 = singles.tile([128, S_DIM], BF16, tag=f"bq0_{b}")
            nc.scalar.activation(out=bq0, in_=bq_ps0, func=AF.Copy)
            bq_ps1 = ps_mm.tile([21, S_DIM], F32, tag="mm")
            for dc in range(6):
                nc.tensor.matmul(bq_ps1, xdl[:, dc, b, 128:149], wqg[:, dc, :],
                                 start=(dc == 0), stop=(dc == 5))
            bq1 = singles.tile([21, S_DIM], BF16, tag=f"bq1_{b}")
            nc.scalar.activation(out=bq1, in_=bq_ps1, func=AF.Copy)

            g_ps0 = ps_mm.tile([128, T_LEN], F32, tag="mm")
            for dc in range(6):
                nc.tensor.matmul(g_ps0, xdl[:, dc, b, 0:128], xdl[:, dc, b, :],
                                 start=(dc == 0), stop=(dc == 5))
            g0 = singles.tile([128, T_LEN], BF16, tag=f"g0_{b}")
            nc.scalar.activation(out=g0, in_=g_ps0, func=AF.Copy)
            g_ps1 = ps_mm.tile([21, T_LEN], F32, tag="mm")
            for dc in range(6):
                nc.tensor.matmul(g_ps1, xdl[:, dc, b, 128:149], xdl[:, dc, b, :],
                                 start=(dc == 0), stop=(dc == 5))
            g1t = singles.tile([21, T_LEN], BF16, tag=f"g1_{b}")
            nc.scalar.activation(out=g1t, in_=g_ps1, func=AF.Copy)
            pre[b] = (sxb0, sxb1, bq0, bq1, g0, g1t)

        # ---- per chunk x batch phases ----
        def phase_load(st):
            b, cols = st["b"], st["cols"]
            ybf = io3.tile([128, 4, CH], BF16, tag="ybf")
            nc.sync.dma_start(out=ybf, in_=din["ybf"][b, :, :, cols])
            st["ybf"] = ybf

        def phase_stats(st):
            b, cols, ybf = st["b"], st["cols"], st["ybf"]
            sxb0, sxb1, bq0, bq1, g0, g1t = pre[b]
            sq2 = wk2.tile([128, 4, CH], BF16, tag="sq")
            nc.scalar.square(out=sq2, in_=ybf)
            # stat segments in SBUF free dim: 0/1 = LN1/LN2 means, 2/3 = E[x^2]
            st_sb = sc.tile([1, 4, CH], F32, tag="stsb")
            s1p = ps_st.tile([1, CH], F32, tag="st")
            nc.tensor.matmul(s1p, sxb0, w1t[:, 0, cols],
                             start=True, stop=False)
            nc.tensor.matmul(s1p, sxb1[:21, :], w1t[:21, 1, cols],
                             start=False, stop=True)
            nc.scalar.activation(out=st_sb[:, 0, :], in_=s1p, func=AF.Copy)
            s2p = ps_st.tile([1, CH], F32, tag="st")
            for m in range(4):
                nc.tensor.matmul(s2p, icol[:, 1:2], ybf[:, m, :],
                                 start=(m == 0), stop=(m == 3))
            nc.scalar.activation(out=st_sb[:, 1, :], in_=s2p, func=AF.Copy)
            # LN1 sumsq via Gram: T = G @ W1T, then colsum(W1T * T)
            t0 = ps_mm.tile([128, CH], F32, tag="mm")
            nc.tensor.matmul(t0, g0[:, 0:128], w1t[:, 0, cols],
                             start=True, stop=False)
            nc.tensor.matmul(t0, g1t[:, 0:128], w1t[:21, 1, cols],
                             start=False, stop=True)
            v0 = wk2.tile([128, CH], BF16, tag="v0")
            nc.vector.tensor_tensor(out=v0, in0=t0, in1=w1t[:, 0, cols],
                                    op=OP.mult)
            t1 = ps_mm.tile([21, CH], F32, tag="mm")
            nc.tensor.matmul(t1, g0[:, 128:149], w1t[:, 0, cols],
                             start=True, stop=False)
            nc.tensor.matmul(t1, g1t[:, 128:149], w1t[:21, 1, cols],
                             start=False, stop=True)
            v1 = wk2.tile([21, CH], BF16, tag="v1")
            nc.vector.tensor_tensor(out=v1, in0=t1, in1=w1t[:21, 1, cols],
                                    op=OP.mult)
            q1p = ps_st.tile([1, CH], F32, tag="st")
            nc.tensor.matmul(q1p, icol[:, 0:1], v0, start=True, stop=False)
            nc.tensor.matmul(q1p, icol[:21, 0:1], v1, start=False, stop=True)
            nc.scalar.activation(out=st_sb[:, 2, :], in_=q1p, func=AF.Copy)
            q2p = ps_st.tile([1, CH], F32, tag="st")
            for m in range(4):
                nc.tensor.matmul(q2p, icol[:, 1:2], sq2[:, m, :],
                                 start=(m == 0), stop=(m == 3))
            nc.scalar.activation(out=st_sb[:, 3, :], in_=q2p, func=AF.Copy)
            st["st_sb"] = st_sb

        def phase_chain(st):
            stp = st["st_sb"]
            mneg = sc.tile([1, 2, CH], BF16, tag="mrow")
            nc.scalar.activation(out=mneg, in_=stp[:, 0:2, :], func=AF.Copy,
                                 scale=-1.0)
            msq = sc.tile([1, 2, CH], F32, tag="msq")
            nc.vector.tensor_mul(out=msq, in0=stp[:, 0:2, :],
                                 in1=stp[:, 0:2, :])
            vrow = sc.tile([1, 2, CH], F32, tag="vrow")
            nc.vector.tensor_tensor(out=vrow, in0=stp[:, 2:4, :], in1=msq,
                                    op=OP.subtract)
            lv = sc.tile([1, 2, CH], F32, tag="lv")
            nc.scalar.activation(out=lv, in_=vrow, func=AF.Ln, bias=eps1)
            rrow = sc.tile([1, 2, CH], BF16, tag="rr")
            nc.scalar.activation(out=rrow, in_=lv, func=AF.Exp, scale=-0.5)
            c2row = sc.tile([1, CH], BF16, tag="c2")
            nc.vector.tensor_mul(out=c2row, in0=mneg[:, 1, :],
                                 in1=rrow[:, 1, :])
            st["mrow"], st["rrow"], st["c2row"] = mneg, rrow, c2row

        def phase_bcast(st):
            rrow = st["rrow"]
            rb = wk2.tile([128, 2, CH], BF16, tag="rb")
            r1b_ps = ps_st.tile([128, CH], F32, tag="bc", bufs=1)
            nc.tensor.matmul(r1b_ps, ones_row, rrow[:, 0, :],
                             start=True, stop=True)
            nc.scalar.activation(out=rb[:, 0, :], in_=r1b_ps, func=AF.Copy)
            r2b_ps = ps_st.tile([128, CH], F32, tag="bc", bufs=1)
            nc.tensor.matmul(r2b_ps, ones_row, rrow[:, 1, :],
                             start=True, stop=True)
            nc.scalar.activation(out=rb[:, 1, :], in_=r2b_ps, func=AF.Copy)
            st["r1b"], st["r2b"] = rb[:, 0, :], rb[:, 1, :]

        def phase_ny(st):
            ybf, r2b, c2row, pe2_t = st["ybf"], st["r2b"], st["c2row"], st["pe2_t"]
            ny = wk2.tile([128, 4, CH], BF16, tag="ny")
            for oc in range(4):
                pc2 = ps_mm.tile([128, CH], F32, tag="mm")
                nc.tensor.matmul(pc2, ones_row, c2row, start=True, stop=False)
                nc.tensor.matmul(pc2, ident, pe2_t[:, oc, :], start=False,
                                 stop=True)
                t1 = att.tile([128, CH], BF16, tag="nyt")
                nc.gpsimd.tensor_mul(out=t1, in0=ybf[:, oc, :], in1=r2b)
                if apply_g2:
                    t2 = att.tile([128, CH], F32, tag="nyt2")
                    nc.vector.tensor_add(out=t2, in0=t1, in1=pc2)
                    nc.scalar.activation(out=ny[:, oc, :], in_=t2,
                                         func=AF.Identity,
                                         scale=g2[:, oc:oc + 1])
                else:
                    nc.vector.tensor_add(out=ny[:, oc, :], in0=t1, in1=pc2)
            st["ny"] = ny

        def phase_q(st):
            b, cols = st["b"], st["cols"]
            mrow, r1b, cq_t = st["mrow"], st["r1b"], st["cq_t"]
            _, _, bq0, bq1, _, _ = pre[b]
            q = wk2.tile([128, 4, CH], BF16, tag="q")
            for oc in range(4):
                ocs = slice(oc * 128, (oc + 1) * 128)
                pq = ps_mm.tile([128, CH], F32, tag="mm")
                nc.tensor.matmul(pq, bq0[:, ocs], w1t[:, 0, cols],
                                 start=True, stop=False)
                nc.tensor.matmul(pq, bq1[:, ocs], w1t[:21, 1, cols],
                                 start=False, stop=False)
                nc.tensor.matmul(pq, uq[:, ocs], mrow[:, 0, :],
                                 start=False, stop=True)
                tmp = att.tile([128, CH], BF16, tag="qt")
                nc.vector.tensor_mul(out=tmp, in0=pq, in1=r1b)
                nc.vector.tensor_add(out=q[:, oc, :], in0=tmp,
                                     in1=cq_t[:, oc, :])
            st["q"] = q

        def phase_k(st):
            ny = st["ny"]
            k = wk2.tile([128, 4, CH], BF16, tag="k")
            for oc in range(4):
                pk = ps_mm.tile([128, CH], F32, tag="mm")
                for kc in range(4):
                    nc.tensor.matmul(
                        pk, wkt[:, kc, oc * 128:(oc + 1) * 128],
                        ny[:, kc, :], start=(kc == 0), stop=(kc == 3))
                nc.vector.tensor_scalar(out=k[:, oc, :], in0=pk,
                                        scalar1=bks[:, oc:oc + 1],
                                        scalar2=None, op0=OP.add)
            st["k"] = k

        def phase_v(st):
            ny = st["ny"]
            v = wk2.tile([112, NBLK, S_DIM], BF16, tag="v")
            for blk in range(NBLK):
                tb = slice(blk * 112, (blk + 1) * 112)
                pt = ps_att.tile([112, S_DIM], BF16, tag="at")
                for co in range(4):
                    nc.tensor.transpose(pt[:, co * 128:(co + 1) * 128],
                                        ny[:, co, tb], ident)
                nc.scalar.activation(out=v[:, blk, :], in_=pt, func=AF.Copy)
            st["v"] = v

        def phase_att(st):
            b, cols = st["b"], st["cols"]
            q, k, v = st["q"], st["k"], st["v"]
            out_t = io2.tile([128, 4, CH], BF16, tag="out")
            for blk in range(NBLK):
                tb = slice(blk * 112, (blk + 1) * 112)
                psc = ps_att.tile([112, 112], F32, tag="at")
                nc.tensor.matmul(psc, ident[:112, :112], masks,
                                 start=True, stop=False)
                for oc in range(4):
                    nc.tensor.matmul(psc, q[:, oc, tb], k[:, oc, tb],
                                     start=False, stop=(oc == 3))
                den = sc.tile([112, 1], F32, tag="den")
                e_b = att.tile([112, 112], BF16, tag="eb")
                nc.scalar.activation(out=e_b, in_=psc, func=AF.Exp,
                                     accum_out=den)
                ld = sc.tile([112, 1], F32, tag="ld")
                nc.scalar.activation(out=ld, in_=den, func=AF.Ln)
                nld = sc.tile([112, 1], F32, tag="nld")
                nc.scalar.activation(out=nld, in_=ld, func=AF.Copy,
                                     scale=-1.0)
                attn = att.tile([112, 112], BF16, tag="attn")
                nc.scalar.activation(out=attn, in_=psc, func=AF.Exp,
                                     bias=nld)
                pat = ps_att.tile([112, 112], BF16, tag="at")
                nc.tensor.transpose(pat, attn, ident[:112, :112])
                attnT = att.tile([112, 112], BF16, tag="attnT")
                nc.vector.tensor_copy(out=attnT, in_=pat)
                pav = ps_att.tile([128, 4, 112], F32, tag="at")
                for co in range(4):
                    nc.tensor.matmul(pav[:, co, :],
                                     v[:, blk, co * 128:(co + 1) * 128],
                                     attnT, start=True, stop=True)
                nc.scalar.activation(out=out_t[:, :, tb], in_=pav,
                                     func=AF.Copy)
            nc.sync.dma_start(out=dout[b, :, :, cols], in_=out_t)

        for ich in range(NCHUNK):
            cols = slice(ich * CH, (ich + 1) * CH)
            cq_t = io2.tile([128, 4, CH], BF16, tag="cq")
            nc.sync.dma_start(out=cq_t, in_=din["cq"][:, :, cols])
            pe2_t = io2.tile([128, 4, CH], BF16, tag="pe2")
            nc.sync.dma_start(out=pe2_t, in_=din["pe2p"][:, :, cols])

            s0 = {"b": 0, "cols": cols, "cq_t": cq_t, "pe2_t": pe2_t}
            s1 = {"b": 1, "cols": cols, "cq_t": cq_t, "pe2_t": pe2_t}

            phase_load(s0)
            phase_load(s1)
            phase_stats(s0)
            phase_chain(s0)
            phase_stats(s1)
            phase_bcast(s0)
            phase_ny(s0)
            phase_chain(s1)
            phase_q(s0)
            phase_bcast(s1)
            phase_k(s0)
            phase_ny(s1)
            phase_v(s0)
            phase_q(s1)
            phase_att(s0)
            phase_k(s1)
            phase_v(s1)
            phase_att(s1)
    return nc


# ----------------------------------------------------------------------------
# Host-side preparation
# ----------------------------------------------------------------------------
def _make_const_inputs(W_conv1, b_conv1, ln1_g, ln1_b, ln2_g, ln2_b,
                       pe_wave, pe_spec, Wq, bq, Wk, bk):
    import ml_dtypes
    f = np.float32
    bf = ml_dtypes.bfloat16
    s = np.float32(S_DIM) ** np.float32(-0.25)

    w1t = np.zeros((128, 2, N_TOK), dtype=f)
    w1T = W_conv1.T.astype(f)
    w1t[:, 0, :] = w1T[:128]
    w1t[:21, 1, :] = w1T[128:]

    # Wqg[d, c] = Wq[c, d] * g1[d] * s, laid out [128, 6, 512]
    wqg = (Wq.T * ln1_g[:, None]).astype(f) * s
    wqg = wqg.reshape(6, 128, S_DIM).transpose(1, 0, 2).copy()
    uq = (Wq @ ln1_g).astype(f)[None, :] * s

    pe_w = pe_wave.reshape(T_DIM, N_TOK).astype(f)
    cq = (Wq @ (ln1_b[:, None] + pe_w)).astype(f) * s + (bq[:, None] * s).astype(f)
    cq = cq.reshape(4, 128, N_TOK).transpose(1, 0, 2).copy()

    # wkt rows scaled by g2 (k-side gamma fold)
    wkt = (Wk.T * (s * ln2_g[:, None])).astype(f)
    wkt = wkt.reshape(4, 128, S_DIM).transpose(1, 0, 2).copy()
    bks = (bk * s).astype(f).reshape(4, 128).T.copy()
    g2 = ln2_g.astype(f).reshape(4, 128).T.copy()
    apply_g2 = not np.allclose(ln2_g, 1.0)

    pe2p = (pe_spec.reshape(S_DIM, N_TOK) + ln2_b[:, None]).astype(f)
    pe2p = pe2p.reshape(4, 128, N_TOK).transpose(1, 0, 2).copy()

    masks = np.full((112, 112), -1e30, dtype=f)
    for sb in range(2):
        masks[sb * 56:(sb + 1) * 56, sb * 56:(sb + 1) * 56] = 0.0

    return {
        "w1t": w1t.astype(bf), "wqg": wqg.astype(bf), "uq": uq.astype(bf),
        "cq": cq.astype(bf), "wkt": wkt.astype(bf), "bks": bks, "g2": g2,
        "pe2p": pe2p.astype(bf), "masks": masks.astype(bf),
        "ones": np.ones((128, 128), dtype=bf),
        "ident": np.eye(128, dtype=bf),
        "icol": np.stack([np.full(128, 1.0 / T_DIM, dtype=f),
                          np.full(128, 1.0 / S_DIM, dtype=f)],
                         axis=1).astype(bf),
        "_apply_g2": apply_g2,
    }


def _make_core_inputs(consts, x_shard, y_shard):
    import ml_dtypes
    bf = ml_dtypes.bfloat16
    x0 = x_shard[:, :128, :].transpose(1, 0, 2).astype(bf).copy()
    x1 = np.zeros((32, B_LOC, T_DIM), dtype=bf)
    x1[:21] = x_shard[:, 128:, :].transpose(1, 0, 2).astype(bf)
    # x_dl[p, dc, b, l] = x[b, l, dc*128+p]
    xdl = x_shard.transpose(2, 0, 1).reshape(6, 128, B_LOC, T_LEN)
    xdl = xdl.transpose(1, 0, 2, 3).astype(bf).copy()
    ybf = y_shard.reshape(B_LOC, 4, 128, N_TOK).transpose(0, 2, 1, 3)
    ybf = ybf.astype(bf).copy()
    m = {"x0": x0, "x1": x1, "xdl": xdl, "ybf": ybf}
    m.update({k: v for k, v in consts.items() if not k.startswith("_")})
    return m


_cached_nc = [None]


def kernel(x, y, W_conv1, b_conv1, ln1_g, ln1_b, ln2_g, ln2_b,
           pe_wave, pe_spec, Wq, bq, Wk, bk):
    _install_patch()
    from concourse.bass_utils import run_bass_kernel_spmd

    x = np.asarray(x, dtype=np.float32)
    y = np.asarray(y, dtype=np.float32)
    consts = _make_const_inputs(
        np.asarray(W_conv1, np.float32), np.asarray(b_conv1, np.float32),
        np.asarray(ln1_g, np.float32), np.asarray(ln1_b, np.float32),
        np.asarray(ln2_g, np.float32), np.asarray(ln2_b, np.float32),
        np.asarray(pe_wave, np.float32), np.asarray(pe_spec, np.float32),
        np.asarray(Wq, np.float32), np.asarray(bq, np.float32),
        np.asarray(Wk, np.float32), np.asarray(bk, np.float32))
    in_maps = [
        _make_core_inputs(consts, x[B_LOC * i:B_LOC * (i + 1)],
                          y[B_LOC * i:B_LOC * (i + 1)])
        for i in range(N_CORES)
    ]

    if _cached_nc[0] is None:
        _cached_nc[0] = _build_program(consts["_apply_g2"])
    nc = _cached_nc[0]

    res = run_bass_kernel_spmd(nc, in_maps, core_ids=list(range(N_CORES)))
    outs = []
    for i in range(N_CORES):
        o = np.asarray(res.results[i]["out"], dtype=np.float32)
        outs.append(o.transpose(0, 2, 1, 3).reshape(B_LOC, S_DIM, H, W))
    return (np.concatenate(outs, axis=0) + y).astype(np.float32)


# revision 42
# speedup vs baseline: 2.0614x; 1.0386x over previous
"""Trainium2 Bass kernel for nn_Expand_36610301231376.

kernel(**inputs) takes the FULL unsharded inputs (as in reference.setup_inputs)
and returns the FULL (16, 512, 56, 56) float32 output.

Strategy: pure data parallel over batch B=16 across 8 NeuronCores (2 batches
per core). Key algebraic restructurings vs a direct implementation:

- The q-projection is composed through the rank-149 conv bottleneck:
  Bq = x @ (diag(g1) Wq^T)  [149, 512] per batch (cheap), then
  q_raw^T = Bq^T routed through W1^T per token chunk (2 matmuls per 128-wide
  output chunk instead of 6), eliminating the xe = conv1(x) materialization.
- LN1 statistics come from the tiny Gram matrix G = x x^T [149,149] and the
  row-sum s_x = sum_d x: sum_d xe = W1 @ s_x, sum_d xe^2 = diag(W1 G W1^T),
  computed per chunk with 8 small matmuls instead of 12 full reductions plus
  squares.
- LN mean/rsqrt chains for both layernorms are batched into one [2,448] pass
  per chunk-batch; reciprocals use the fast approximate DVE op.
- The conv bias b_conv1 cancels in LN; LN gammas/betas, positional encodings,
  projection biases and the 1/sqrt(512) attention scale are folded into
  host-precomputed constants.
- The residual +y is applied on the host; the device returns bf16 attention
  output, halving output DMA.
"""
import sys

if "/opt/trn_rl_repo" not in sys.path:
    sys.path.insert(0, "/opt/trn_rl_repo")

import numpy as np
import orjson

# ----------------------------------------------------------------------------
# BIR post-pass: this container's walrus build supports only ONE sync-wait per
# instruction; split multi-wait instructions into single-wait NoOps.
# ----------------------------------------------------------------------------
_wcounter = [0]


def _split_block(instructions):
    out, changed = [], False
    for inst in instructions:
        si = inst.get("sync_info")
        waits = (si or {}).get("on_wait") or []
        if len(waits) > 1:
            changed = True
            for w in waits[:-1]:
                _wcounter[0] += 1
                nop = {
                    "engine": inst["engine"], "ins": [], "outs": [],
                    "name": f"I-wsplit-{_wcounter[0]}", "opcode": "NoOp",
                    "sync_info": {"on_update": [], "on_wait": [w]},
                }
                if "debug" in inst:
                    nop["debug"] = inst["debug"]
                out.append(nop)
            si["on_wait"] = [waits[-1]]
        out.append(inst)
    return out, changed


def _split_multi_waits_json(bir_json: bytes) -> bytes:
    m = orjson.loads(bir_json)
    changed = False
    for fn in m.get("functions", []):
        for blk in fn.get("blocks", []):
            insts = blk.get("instructions")
            if insts:
                blk["instructions"], ch = _split_block(insts)
                changed = changed or ch
    return orjson.dumps(m) if changed else bir_json


def _install_patch():
    import concourse.bass as bass

    if getattr(bass.Bass, "_wait_split_installed", False):
        return
    orig = bass.Bass.to_json_bytes

    def to_json_bytes(self):
        return _split_multi_waits_json(orig(self))

    bass.Bass.to_json_bytes = to_json_bytes
    bass.Bass._wait_split_installed = True


# ----------------------------------------------------------------------------
# Problem constants (hardcoded from the problem spec)
# ----------------------------------------------------------------------------
B = 16
N_CORES = 8
B_LOC = B // N_CORES
T_LEN, T_DIM = 149, 768
H = W = 56
S_DIM = 512
N_TOK = H * W           # 3136
CH = 448                # tokens per chunk (8 image rows)
NCHUNK = N_TOK // CH    # 7
NBLK = CH // 112        # 4 two-row attention blocks per chunk
EPS = 1e-5


# ----------------------------------------------------------------------------
# Device program
# ----------------------------------------------------------------------------
def _build_program(apply_g2: bool):
    import concourse.bass as bass
    import concourse.tile as tile
    from concourse import mybir

    F32 = mybir.dt.float32
    BF16 = mybir.dt.bfloat16
    AF = mybir.ActivationFunctionType
    OP = mybir.AluOpType

    nc = bass.Bass(trn_type="TRN2", target_bir_lowering=False, debug=False)
    din = {}
    for name, shape, dt_ in [
        ("x0", (128, B_LOC, T_DIM), BF16), ("x1", (32, B_LOC, T_DIM), BF16),
        ("xdl", (128, 6, B_LOC, T_LEN), BF16),
        ("wqg", (128, 6, S_DIM), BF16),
        ("w1t", (128, 2, N_TOK), BF16),
        ("uq", (1, S_DIM), BF16), ("uk", (1, S_DIM), BF16),
        ("wkt", (128, 4, S_DIM), BF16),
        ("ones", (128, 128), BF16), ("ident", (128, 128), BF16),
        ("cq", (128, 4, N_TOK), BF16), ("ck", (128, 4, N_TOK), BF16),
        ("pe2t", (112, NCHUNK, 4, S_DIM), BF16),
        ("g2bt", (112, S_DIM), BF16),
        ("masks", (112, 112), BF16),
        ("icol", (128, 2), BF16),
        ("ybf", (B_LOC, 128, 4, N_TOK), BF16),
    ]:
        din[name] = nc.dram_tensor(name, list(shape), dt_, kind="ExternalInput").ap()
    dout = nc.dram_tensor("out", [B_LOC, 128, 4, N_TOK], BF16,
                          kind="ExternalOutput").ap()

    from contextlib import ExitStack

    with nc.allow_low_precision(reason="bf16 matmul operands, fp32 accumulate"), \
         tile.TileContext(nc) as tc, ExitStack() as ctx:
        singles = ctx.enter_context(tc.tile_pool(name="singles", bufs=1))
        io3 = ctx.enter_context(tc.tile_pool(name="io3", bufs=3))
        io2 = ctx.enter_context(tc.tile_pool(name="io2", bufs=2))
        wk2 = ctx.enter_context(tc.tile_pool(name="wk2", bufs=2))
        att = ctx.enter_context(tc.tile_pool(name="att", bufs=3))
        sc = ctx.enter_context(tc.tile_pool(name="sc", bufs=2))
        ps_mm = ctx.enter_context(tc.tile_pool(name="ps_mm", bufs=3, space="PSUM"))
        ps_st = ctx.enter_context(tc.tile_pool(name="ps_st", bufs=2, space="PSUM"))
        ps_att = ctx.enter_context(tc.tile_pool(name="ps_att", bufs=2, space="PSUM"))

        def load(name, shape, dt_):
            t = singles.tile(list(shape), dt_, tag=name)
            nc.sync.dma_start(out=t, in_=din[name])
            return t

        x0 = load("x0", (128, B_LOC, T_DIM), BF16)
        x1 = load("x1", (32, B_LOC, T_DIM), BF16)
        xdl = load("xdl", (128, 6, B_LOC, T_LEN), BF16)
        wqg = load("wqg", (128, 6, S_DIM), BF16)
        w1t = load("w1t", (128, 2, N_TOK), BF16)
        uq = load("uq", (1, S_DIM), BF16)
        uk = load("uk", (1, S_DIM), BF16)
        wkt = load("wkt", (128, 4, S_DIM), BF16)
        ones = load("ones", (128, 128), BF16)
        ident = load("ident", (128, 128), BF16)
        g2bt = load("g2bt", (112, S_DIM), BF16)
        masks = load("masks", (112, 112), BF16)
        ones_col = ones[:, 0:1]
        ones_row = ones[0:1, :]
        eps1 = singles.tile([1, 1], F32)
        nc.vector.memset(eps1, EPS)
        onef = singles.tile([1, 1], F32)
        nc.vector.memset(onef, 1.0)
        icol = load("icol", (128, 2), BF16)

        # ---- per-batch preamble: s_x, Bq = x @ Wqg, G = x x^T ----
        pre = {}
        for b in range(B_LOC):
            sx0 = singles.tile([128, 1], F32, tag=f"sx0_{b}")
            nc.vector.tensor_reduce(out=sx0, in_=x0[:, b, :],
                                    axis=mybir.AxisListType.X, op=OP.add)
            sx1 = singles.tile([32, 1], F32, tag=f"sx1_{b}")
            nc.vector.tensor_reduce(out=sx1, in_=x1[:, b, :],
                                    axis=mybir.AxisListType.X, op=OP.add)
            sxb0 = singles.tile([128, 1], BF16, tag=f"sxb0_{b}")
            nc.scalar.activation(out=sxb0, in_=sx0, func=AF.Copy,
                                 scale=1.0 / T_DIM)
            sxb1 = singles.tile([32, 1], BF16, tag=f"sxb1_{b}")
            nc.scalar.activation(out=sxb1, in_=sx1, func=AF.Copy,
                                 scale=1.0 / T_DIM)

            bq_ps0 = ps_mm.tile([128, S_DIM], F32, tag="mm")
            for dc in range(6):
                nc.tensor.matmul(bq_ps0, xdl[:, dc, b, 0:128], wqg[:, dc, :],
                                 start=(dc == 0), stop=(dc == 5))
            bq0 = singles.tile([128, S_DIM], BF16, tag=f"bq0_{b}")
            nc.scalar.activation(out=bq0, in_=bq_ps0, func=AF.Copy)
            bq_ps1 = ps_mm.tile([21, S_DIM], F32, tag="mm")
            for dc in range(6):
                nc.tensor.matmul(bq_ps1, xdl[:, dc, b, 128:149], wqg[:, dc, :],
                                 start=(dc == 0), stop=(dc == 5))
            bq1 = singles.tile([21, S_DIM], BF16, tag=f"bq1_{b}")
            nc.scalar.activation(out=bq1, in_=bq_ps1, func=AF.Copy)

            g_ps0 = ps_mm.tile([128, T_LEN], F32, tag="mm")
            for dc in range(6):
                nc.tensor.matmul(g_ps0, xdl[:, dc, b, 0:128], xdl[:, dc, b, :],
                                 start=(dc == 0), stop=(dc == 5))
            g0 = singles.tile([128, T_LEN], BF16, tag=f"g0_{b}")
            nc.scalar.activation(out=g0, in_=g_ps0, func=AF.Copy)
            g_ps1 = ps_mm.tile([21, T_LEN], F32, tag="mm")
            for dc in range(6):
                nc.tensor.matmul(g_ps1, xdl[:, dc, b, 128:149], xdl[:, dc, b, :],
                                 start=(dc == 0), stop=(dc == 5))
            g1t = singles.tile([21, T_LEN], BF16, tag=f"g1_{b}")
            nc.scalar.activation(out=g1t, in_=g_ps1, func=AF.Copy)
            pre[b] = (sxb0, sxb1, bq0, bq1, g0, g1t)

        # ---- per chunk x batch phases, software-pipelined over units ----
        def em_ld(u):
            if u["first"]:
                cq_t = io2.tile([128, 4, CH], BF16, tag="cq", bufs=3)
                nc.sync.dma_start(out=cq_t, in_=din["cq"][:, :, u["cols"]])
                ck_t = io2.tile([128, 4, CH], BF16, tag="ck", bufs=3)
                nc.sync.dma_start(out=ck_t, in_=din["ck"][:, :, u["cols"]])
                pe2_t = io2.tile([112, 4, S_DIM], BF16, tag="pe2", bufs=3)
                nc.sync.dma_start(out=pe2_t, in_=din["pe2t"][:, u["ich"], :, :])
                chunk_io[u["ich"]] = (cq_t, ck_t, pe2_t)
            u["cq_t"], u["ck_t"], u["pe2_t"] = chunk_io[u["ich"]]
            ybf = io3.tile([128, 4, CH], BF16, tag="ybf")
            nc.sync.dma_start(out=ybf, in_=din["ybf"][u["b"], :, :, u["cols"]])
            u["ybf"] = ybf

        def em_stats(u):
            b, cols, ybf = u["b"], u["cols"], u["ybf"]
            sxb0, sxb1, bq0, bq1, g0, g1t = pre[b]
            sq2 = wk2.tile([128, 4, CH], BF16, tag="sq")
            nc.scalar.square(out=sq2, in_=ybf)
            # stat segments in SBUF free dim: 0/1 = LN1/LN2 means, 2/3 = E[x^2]
            st_sb = sc.tile([1, 4, CH], F32, tag="stsb")
            s1p = ps_st.tile([1, CH], F32, tag="st")
            nc.tensor.matmul(s1p, sxb0, w1t[:, 0, cols],
                             start=True, stop=False)
            nc.tensor.matmul(s1p, sxb1[:21, :], w1t[:21, 1, cols],
                             start=False, stop=True)
            nc.scalar.activation(out=st_sb[:, 0, :], in_=s1p, func=AF.Copy)
            s2p = ps_st.tile([1, CH], F32, tag="st")
            for m in range(4):
                nc.tensor.matmul(s2p, icol[:, 1:2], ybf[:, m, :],
                                 start=(m == 0), stop=(m == 3))
            nc.scalar.activation(out=st_sb[:, 1, :], in_=s2p, func=AF.Copy)
            # LN1 sumsq via Gram: T = G @ W1T, then colsum(W1T * T)
            t0 = ps_mm.tile([128, CH], F32, tag="mm")
            nc.tensor.matmul(t0, g0[:, 0:128], w1t[:, 0, cols],
                             start=True, stop=False)
            nc.tensor.matmul(t0, g1t[:, 0:128], w1t[:21, 1, cols],
                             start=False, stop=True)
            v0 = wk2.tile([128, CH], BF16, tag="v0")
            nc.vector.tensor_tensor(out=v0, in0=t0, in1=w1t[:, 0, cols],
                                    op=OP.mult)
            t1 = ps_mm.tile([21, CH], F32, tag="mm")
            nc.tensor.matmul(t1, g0[:, 128:149], w1t[:, 0, cols],
                             start=True, stop=False)
            nc.tensor.matmul(t1, g1t[:, 128:149], w1t[:21, 1, cols],
                             start=False, stop=True)
            v1 = wk2.tile([21, CH], BF16, tag="v1")
            nc.vector.tensor_tensor(out=v1, in0=t1, in1=w1t[:21, 1, cols],
                                    op=OP.mult)
            q1p = ps_st.tile([1, CH], F32, tag="st")
            nc.tensor.matmul(q1p, icol[:, 0:1], v0, start=True, stop=False)
            nc.tensor.matmul(q1p, icol[:21, 0:1], v1, start=False, stop=True)
            nc.scalar.activation(out=st_sb[:, 2, :], in_=q1p, func=AF.Copy)
            q2p = ps_st.tile([1, CH], F32, tag="st")
            for m in range(4):
                nc.tensor.matmul(q2p, icol[:, 1:2], sq2[:, m, :],
                                 start=(m == 0), stop=(m == 3))
            nc.scalar.activation(out=st_sb[:, 3, :], in_=q2p, func=AF.Copy)
            u["st_sb"] = st_sb

        def em_chain(u):
            stp = u["st_sb"]
            mneg = sc.tile([1, 2, CH], BF16, tag="mrow")
            nc.scalar.activation(out=mneg, in_=stp[:, 0:2, :], func=AF.Copy,
                                 scale=-1.0)
            msq = sc.tile([1, 2, CH], F32, tag="msq")
            nc.vector.tensor_mul(out=msq, in0=stp[:, 0:2, :],
                                 in1=stp[:, 0:2, :])
            vrow = sc.tile([1, 2, CH], F32, tag="vrow")
            nc.vector.tensor_tensor(out=vrow, in0=stp[:, 2:4, :], in1=msq,
                                    op=OP.subtract)
            lv = sc.tile([1, 2, CH], F32, tag="lv")
            nc.scalar.activation(out=lv, in_=vrow, func=AF.Ln, bias=eps1)
            rrow_f = sc.tile([1, 2, CH], F32, tag="rrf")
            nc.scalar.activation(out=rrow_f, in_=lv, func=AF.Exp, scale=-0.5)
            rrow = sc.tile([1, 2, CH], BF16, tag="rr")
            nc.scalar.activation(out=rrow, in_=rrow_f, func=AF.Copy)
            c2row = sc.tile([1, CH], F32, tag="c2")
            nc.vector.tensor_mul(out=c2row, in0=mneg[:, 1, :],
                                 in1=rrow_f[:, 1, :])
            u["mneg"], u["rrow"], u["c2row"] = mneg, rrow, c2row
            u["rrow_f"] = rrow_f

        def em_bcast(u):
            rrow, c2row = u["rrow"], u["c2row"]
            rb = wk2.tile([128, 2, CH], BF16, tag="rb")
            r1b_ps = ps_st.tile([128, CH], F32, tag="bc", bufs=1)
            nc.tensor.matmul(r1b_ps, ones_row, rrow[:, 0, :],
                             start=True, stop=True)
            nc.scalar.activation(out=rb[:, 0, :], in_=r1b_ps, func=AF.Copy)
            r2b_ps = ps_st.tile([128, CH], F32, tag="bc", bufs=1)
            nc.tensor.matmul(r2b_ps, ones_row, rrow[:, 1, :],
                             start=True, stop=True)
            nc.scalar.activation(out=rb[:, 1, :], in_=r2b_ps, func=AF.Copy)
            u["r1b"], u["r2b"] = rb[:, 0, :], rb[:, 1, :]
            # token-major LN2 stat columns (rsqrt, -mean*rsqrt) per block
            rrow_f, c2row = u["rrow_f"], u["c2row"]
            rcp = ps_st.tile([112, 8], F32, tag="st")
            for blk in range(NBLK):
                tb = slice(blk * 112, (blk + 1) * 112)
                nc.tensor.transpose(rcp[:, 2 * blk:2 * blk + 1],
                                    rrow_f[:, 1, tb], onef)
                nc.tensor.transpose(rcp[:, 2 * blk + 1:2 * blk + 2],
                                    c2row[:, tb], onef)
            rc = sc.tile([112, 8], F32, tag="rc")
            nc.scalar.activation(out=rc, in_=rcp, func=AF.Copy)
            u["rc"] = rc

        def em_q(u):
            b, cols = u["b"], u["cols"]
            mneg, r1b, cq_t = u["mneg"], u["r1b"], u["cq_t"]
            _, _, bq0, bq1, _, _ = pre[b]
            q = wk2.tile([128, 4, CH], BF16, tag="q")
            for oc in range(4):
                ocs = slice(oc * 128, (oc + 1) * 128)
                pq = ps_mm.tile([128, CH], F32, tag="mm")
                nc.tensor.matmul(pq, bq0[:, ocs], w1t[:, 0, cols],
                                 start=True, stop=False)
                nc.tensor.matmul(pq, bq1[:, ocs], w1t[:21, 1, cols],
                                 start=False, stop=False)
                nc.tensor.matmul(pq, uq[:, ocs], mneg[:, 0, :],
                                 start=False, stop=True)
                tmp = att.tile([128, CH], BF16, tag="qt")
                nc.vector.tensor_mul(out=tmp, in0=pq, in1=r1b)
                nc.vector.tensor_add(out=q[:, oc, :], in0=tmp,
                                     in1=cq_t[:, oc, :])
            u["q"] = q

        def em_k(u):
            ybf, mneg, r2b, ck_t = u["ybf"], u["mneg"], u["r2b"], u["ck_t"]
            k = wk2.tile([128, 4, CH], BF16, tag="k")
            for oc in range(4):
                ocs = slice(oc * 128, (oc + 1) * 128)
                pk = ps_mm.tile([128, CH], F32, tag="mm")
                for kc in range(4):
                    nc.tensor.matmul(pk, wkt[:, kc, ocs], ybf[:, kc, :],
                                     start=(kc == 0), stop=False)
                nc.tensor.matmul(pk, uk[:, ocs], mneg[:, 1, :],
                                 start=False, stop=True)
                kt = att.tile([128, CH], BF16, tag="kt")
                nc.vector.tensor_mul(out=kt, in0=pk, in1=r2b)
                nc.vector.tensor_add(out=k[:, oc, :], in0=kt,
                                     in1=ck_t[:, oc, :])
            u["k"] = k

        def em_v(u):
            ybf, rc, pe2_t = u["ybf"], u["rc"], u["pe2_t"]
            v = wk2.tile([112, NBLK, S_DIM], BF16, tag="v")
            for blk in range(NBLK):
                tb = slice(blk * 112, (blk + 1) * 112)
                pt = ps_att.tile([112, S_DIM], BF16, tag="at")
                for co in range(4):
                    nc.tensor.transpose(pt[:, co * 128:(co + 1) * 128],
                                        ybf[:, co, tb], ident)
                v1t = att.tile([112, S_DIM], BF16, tag="v1t")
                nc.vector.tensor_scalar(out=v1t, in0=pt,
                                        scalar1=rc[:, 2 * blk:2 * blk + 1],
                                        scalar2=rc[:, 2 * blk + 1:2 * blk + 2],
                                        op0=OP.mult, op1=OP.add)
                if apply_g2:
                    v1g = att.tile([112, S_DIM], BF16, tag="v1g")
                    nc.vector.tensor_mul(out=v1g, in0=v1t, in1=g2bt)
                    v1t = v1g
                nc.gpsimd.tensor_add(out=v[:, blk, :], in0=v1t,
                                     in1=pe2_t[:, blk, :])
            u["v"] = v

        def em_att(u):
            b, cols = u["b"], u["cols"]
            q, k, v = u["q"], u["k"], u["v"]
            out_t = io2.tile([128, 4, CH], BF16, tag="out")
            for blk in range(NBLK):
                tb = slice(blk * 112, (blk + 1) * 112)
                psc = ps_att.tile([112, 112], F32, tag="at")
                nc.tensor.matmul(psc, ident[:112, :112], masks,
                                 start=True, stop=False)
                for oc in range(4):
                    nc.tensor.matmul(psc, q[:, oc, tb], k[:, oc, tb],
                                     start=False, stop=(oc == 3))
                den = sc.tile([112, 1], F32, tag="den")
                e_b = att.tile([112, 112], BF16, tag="eb")
                nc.scalar.activation(out=e_b, in_=psc, func=AF.Exp,
                                     accum_out=den)
                ld = sc.tile([112, 1], F32, tag="ld")
                nc.scalar.activation(out=ld, in_=den, func=AF.Ln)
                nld = sc.tile([112, 1], F32, tag="nld")
                nc.scalar.activation(out=nld, in_=ld, func=AF.Copy,
                                     scale=-1.0)
                attn = att.tile([112, 112], BF16, tag="attn")
                nc.scalar.activation(out=attn, in_=psc, func=AF.Exp,
                                     bias=nld)
                pat = ps_att.tile([112, 112], BF16, tag="at")
                nc.tensor.transpose(pat, attn, ident[:112, :112])
                attnT = att.tile([112, 112], BF16, tag="attnT")
                nc.vector.tensor_copy(out=attnT, in_=pat)
                pav = ps_att.tile([128, 4, 112], F32, tag="at")
                for co in range(4):
                    nc.tensor.matmul(pav[:, co, :],
                                     v[:, blk, co * 128:(co + 1) * 128],
                                     attnT, start=True, stop=True)
                nc.scalar.activation(out=out_t[:, :, tb], in_=pav,
                                     func=AF.Copy)
            nc.sync.dma_start(out=dout[b, :, :, cols], in_=out_t)

        units = []
        for ich in range(NCHUNK):
            cols = slice(ich * CH, (ich + 1) * CH)
            for b in range(B_LOC):
                units.append({"b": b, "ich": ich, "cols": cols,
                              "first": b == 0})
        chunk_io = {}
        n = len(units)
        for i in range(-3, n):
            if 0 <= i < n:
                em_att(units[i])
            if 0 <= i + 1 < n:
                em_q(units[i + 1])
                em_k(units[i + 1])
                em_v(units[i + 1])
            if 0 <= i + 2 < n:
                em_stats(units[i + 2])
                em_chain(units[i + 2])
                em_bcast(units[i + 2])
            if 0 <= i + 3 < n:
                em_ld(units[i + 3])
    return nc


# ----------------------------------------------------------------------------
# Host-side preparation
# ----------------------------------------------------------------------------
def _make_const_inputs(W_conv1, b_conv1, ln1_g, ln1_b, ln2_g, ln2_b,
                       pe_wave, pe_spec, Wq, bq, Wk, bk):
    import ml_dtypes
    f = np.float32
    bf = ml_dtypes.bfloat16
    s = np.float32(S_DIM) ** np.float32(-0.25)

    w1t = np.zeros((128, 2, N_TOK), dtype=f)
    w1T = W_conv1.T.astype(f)
    w1t[:, 0, :] = w1T[:128]
    w1t[:21, 1, :] = w1T[128:]

    # Wqg[d, c] = Wq[c, d] * g1[d] * s, laid out [128, 6, 512]
    wqg = (Wq.T * ln1_g[:, None]).astype(f) * s
    wqg = wqg.reshape(6, 128, S_DIM).transpose(1, 0, 2).copy()
    uq = (Wq @ ln1_g).astype(f)[None, :] * s

    pe_w = pe_wave.reshape(T_DIM, N_TOK).astype(f)
    cq = (Wq @ (ln1_b[:, None] + pe_w)).astype(f) * s + (bq[:, None] * s).astype(f)
    cq = cq.reshape(4, 128, N_TOK).transpose(1, 0, 2).copy()

    # wkt rows scaled by g2 (k-side gamma fold)
    wkt = (Wk.T * (s * ln2_g[:, None])).astype(f)
    wkt = wkt.reshape(4, 128, S_DIM).transpose(1, 0, 2).copy()
    uk = (Wk @ ln2_g).astype(f)[None, :] * s
    apply_g2 = not np.allclose(ln2_g, 1.0)

    pe2_full = (pe_spec.reshape(S_DIM, N_TOK) + ln2_b[:, None]).astype(f)
    ck = ((Wk * ln2_g[None, :]) @ pe2_full) * s + (bk * s)[:, None]
    ck = ck.astype(f).reshape(4, 128, N_TOK).transpose(1, 0, 2).copy()
    pe2t = pe2_full.reshape(S_DIM, NCHUNK, NBLK, 112)
    pe2t = pe2t.transpose(3, 1, 2, 0).copy()
    g2bt = np.broadcast_to(ln2_g[None, :].astype(f), (112, S_DIM)).copy()

    masks = np.full((112, 112), -1e30, dtype=f)
    for sb in range(2):
        masks[sb * 56:(sb + 1) * 56, sb * 56:(sb + 1) * 56] = 0.0

    return {
        "w1t": w1t.astype(bf), "wqg": wqg.astype(bf), "uq": uq.astype(bf),
        "uk": uk.astype(bf), "cq": cq.astype(bf), "wkt": wkt.astype(bf),
        "ck": ck.astype(bf), "pe2t": pe2t.astype(bf),
        "g2bt": g2bt.astype(bf), "masks": masks.astype(bf),
        "ones": np.ones((128, 128), dtype=bf),
        "ident": np.eye(128, dtype=bf),
        "icol": np.stack([np.full(128, 1.0 / T_DIM, dtype=f),
                          np.full(128, 1.0 / S_DIM, dtype=f)],
                         axis=1).astype(bf),
        "_apply_g2": apply_g2,
    }


def _make_core_inputs(consts, x_shard, y_shard):
    import ml_dtypes
    bf = ml_dtypes.bfloat16
    x0 = x_shard[:, :128, :].transpose(1, 0, 2).astype(bf).copy()
    x1 = np.zeros((32, B_LOC, T_DIM), dtype=bf)
    x1[:21] = x_shard[:, 128:, :].transpose(1, 0, 2).astype(bf)
    # x_dl[p, dc, b, l] = x[b, l, dc*128+p]
    xdl = x_shard.transpose(2, 0, 1).reshape(6, 128, B_LOC, T_LEN)
    xdl = xdl.transpose(1, 0, 2, 3).astype(bf).copy()
    ybf = y_shard.reshape(B_LOC, 4, 128, N_TOK).transpose(0, 2, 1, 3)
    ybf = ybf.astype(bf).copy()
    m = {"x0": x0, "x1": x1, "xdl": xdl, "ybf": ybf}
    m.update({k: v for k, v in consts.items() if not k.startswith("_")})
    return m


_cached_nc = [None]


def kernel(x, y, W_conv1, b_conv1, ln1_g, ln1_b, ln2_g, ln2_b,
           pe_wave, pe_spec, Wq, bq, Wk, bk):
    _install_patch()
    from concourse.bass_utils import run_bass_kernel_spmd

    x = np.asarray(x, dtype=np.float32)
    y = np.asarray(y, dtype=np.float32)
    consts = _make_const_inputs(
        np.asarray(W_conv1, np.float32), np.asarray(b_conv1, np.float32),
        np.asarray(ln1_g, np.float32), np.asarray(ln1_b, np.float32),
        np.asarray(ln2_g, np.float32), np.asarray(ln2_b, np.float32),
        np.asarray(pe_wave, np.float32), np.asarray(pe_spec, np.float32),
        np.asarray(Wq, np.float32), np.asarray(bq, np.float32),
        np.asarray(Wk, np.float32), np.asarray(bk, np.float32))
    in_maps = [
        _make_core_inputs(consts, x[B_LOC * i:B_LOC * (i + 1)],
                          y[B_LOC * i:B_LOC * (i + 1)])
        for i in range(N_CORES)
    ]

    if _cached_nc[0] is None:
        _cached_nc[0] = _build_program(consts["_apply_g2"])
    nc = _cached_nc[0]

    res = run_bass_kernel_spmd(nc, in_maps, core_ids=list(range(N_CORES)))
    outs = []
    for i in range(N_CORES):
        o = np.asarray(res.results[i]["out"], dtype=np.float32)
        outs.append(o.transpose(0, 2, 1, 3).reshape(B_LOC, S_DIM, H, W))
    return (np.concatenate(outs, axis=0) + y).astype(np.float32)
